# revision 14
# baseline (speedup 1.0000x reference)
"""CKConv (SIREN-generated causal conv1d) Trainium2 kernel.

Problem: x[B=4, Cin=32, L=2048]; a tiny SIREN MLP generates a conv kernel
[Cout=32, Cin=32, L]; output = causal conv + bias -> [4, 32, 2048].

Strategy:
  - Host: run the (negligible, O(H*L)) SIREN generator in numpy with
    REVERSED positions, producing the flipped kernel Wf[o,i,j'] directly
    (out[b,o,t] = sum_{i,j'<=t} Wf[o,i,j'] * x[b,i,t-j']), and pre-shuffle
    it into matmul tile layout.
  - Device (8 NeuronCores, SPMD): core k handles batch b=k//2 and the
    tap-parity half h=k%2 (16-tap blocks with block index = h mod 2);
    the two cores of a pair produce partial sums the host adds (2M flops).
    The causal conv is computed as dense 128x128x512 TensorE matmuls in
    float32r:
      K = 128 = (dj in 4) x (i in 32)   -- im2col: 4 time-shifted x copies
      M = 128 = (g in 4) x (o in 32)    -- 4 tap-groups per call
      N = 512                           -- one PSUM bank
    Local call l covers taps j' = 32l + 16h + 4g + dj. For output tile T
    (512 cols), local calls l=0..16(T+1)-1 accumulate in PSUM (causality:
    later taps only matter for later tiles). Tap-group g lands shifted by
    4g columns; VectorE folds the 4 groups (32-partition aligned blocks)
    into a [32, 2048] accumulator, ScalarE adds bias (h=0 core only),
    DMA out.
"""

import numpy as np

import concourse.mybir as mybir
import concourse.tile as tile
from concourse import bacc
from concourse.bass_utils import run_bass_kernel_spmd

B, CIN, COUT, L, HID = 4, 32, 32, 2048, 32
OMEGA = 30.0
NCORES = 8
PAD = 512            # xim left zero padding (covers max lookback 16*31+496)
XIMW = PAD + L       # 2560
NT = 4               # output tiles of 512
NCH = 64             # weight tiles (local calls) per core; 16 taps each

TRACE = False
LAST_EXEC_NS = None
LAST_RESULTS = None

_NC = None


def _build_nc():
    nc = bacc.Bacc(None, target_bir_lowering=False)
    f32 = mybir.dt.float32
    f32r = mybir.dt.float32r
    # x pre-padded host-side: xin[i, 4-dj : 4-dj+? ] windows cover all of
    # xim with zeros on the left, so no memset is needed (fp32r typing).
    xin = nc.dram_tensor("xin", [CIN, 4 + PAD + L], f32r, kind="ExternalInput")
    wd = nc.dram_tensor("w", [128, NCH, 128], f32r, kind="ExternalInput")
    bd = nc.dram_tensor("bias", [COUT, 1], f32, kind="ExternalInput")
    od = nc.dram_tensor("out", [COUT, L], f32, kind="ExternalOutput")

    with tile.TileContext(nc) as tc:
        with (
            tc.tile_pool(name="const", bufs=1) as cpool,
            tc.tile_pool(name="ps", bufs=2, space="PSUM") as pspool,
            tc.tile_pool(name="stage", bufs=2) as spool,
        ):
            # im2col input: xim[dj*32 + i, PAD + t + dj] = x[i, t]
            # xin host layout: [4 zero cols | PAD zero cols | x]; the dj-th
            # block reads xin[:, 4-dj : 4-dj+XIMW] so xim[dj*32+i, s] =
            # x[i, s - PAD - dj] with zeros for s < PAD + dj.
            xim = cpool.tile([128, XIMW], f32r)
            for dj in range(4):
                nc.sync.dma_start(
                    out=xim[dj * 32 : (dj + 1) * 32, :],
                    in_=xin[:, 4 - dj : 4 - dj + XIMW],
                )

            # weight tiles, 4 chunks of 16 calls for DMA/compute overlap
            wch = []
            for t in range(4):
                wt = cpool.tile([128, 16, 128], f32r, tag=f"w{t}")
                nc.sync.dma_start(out=wt[:], in_=wd[:, 16 * t : 16 * (t + 1), :])
                wch.append(wt)

            bias_sb = cpool.tile([COUT, 1], f32)
            nc.sync.dma_start(out=bias_sb[:], in_=bd[:])

            acc = cpool.tile([COUT, L + 64], f32)
            nc.vector.memset(acc[:], 0.0)

            for T in range(NT):
                ps = pspool.tile([128, 512], f32)
                ncalls = 16 * (T + 1)
                for l in range(ncalls):
                    s = PAD + 512 * T - 32 * l
                    nc.tensor.matmul(
                        ps[:],
                        wch[l // 16][:, l % 16, :],
                        xim[:, s : s + 512],
                        start=(l == 0),
                        stop=(l == ncalls - 1),
                    )
                # fold tap-groups: psum[g*32+o, n] -> out[o, 512T + n + 4g]
                for g in range(4):
                    nc.vector.tensor_add(
                        out=acc[:, 512 * T + 4 * g : 512 * T + 512],
                        in0=acc[:, 512 * T + 4 * g : 512 * T + 512],
                        in1=ps[32 * g : 32 * g + 32, 0 : 512 - 4 * g],
                    )
                for g in range(1, 4):
                    nc.vector.tensor_add(
                        out=acc[:, 512 * (T + 1) : 512 * (T + 1) + 4 * g],
                        in0=acc[:, 512 * (T + 1) : 512 * (T + 1) + 4 * g],
                        in1=ps[32 * g : 32 * g + 32, 512 - 4 * g : 512],
                    )
                ft = spool.tile([COUT, 512], f32)
                nc.scalar.activation(
                    ft[:],
                    acc[:, 512 * T : 512 * T + 512],
                    mybir.ActivationFunctionType.Identity,
                    bias=bias_sb[:],
                )
                nc.sync.dma_start(out=od[:, 512 * T : 512 * T + 512], in_=ft[:])

    nc.compile()
    return nc


def _gen_flipped_kernel(w1, b1, w2, b2, w3, b3):
    """SIREN generator with reversed positions -> Wf[o, i, j'] = k[o, i, L-1-j']."""
    pos = np.linspace(-1.0, 1.0, L, dtype=np.float32)[::-1].astype(np.float64)
    w1 = w1.astype(np.float64)
    w2 = w2.astype(np.float64)
    w3 = w3.astype(np.float64)
    h = np.sin(OMEGA * (w1[:, 0][:, None] * pos[None, :] + b1.astype(np.float64)[:, None]))
    h = np.sin(OMEGA * (w2 @ h + b2.astype(np.float64)[:, None]))
    k = w3 @ h + b3.astype(np.float64)[:, None]
    return k.reshape(COUT, CIN, L).astype(np.float32)


def _shuffle_weights(wf):
    """wf[o,i,j'] -> per tap-parity-half h: wt[p=dj*32+i, l, m=g*32+o]
    = wf[o, i, 32l + 16h + 4g + dj].

    The device pairs call l's weights with x-window column
    512T + n - 32l - dj and the unpack maps psum col n to
    out t = 512T + n + 4g.  With tap J = 32l + 16h + 4g + dj the correct
    x index is t - J = 512T + n - 32l - dj - 16h: the h=1 core therefore
    receives its input shifted right by 16 columns (see kernel()), which
    makes the device program identical on all cores.
    """
    outs = []
    for h in range(2):
        v = wf.reshape(COUT, CIN, NCH, 2, 4, 4)[:, :, :, h]   # [o,i,l,g,dj]
        v = v.transpose(4, 1, 2, 3, 0)                        # [dj,i,l,g,o]
        outs.append(np.ascontiguousarray(v.reshape(128, NCH, 128), dtype=np.float32))
    return outs


def kernel(x, w1, b1, w2, b2, w3, b3, bias):
    global _NC, LAST_EXEC_NS, LAST_RESULTS
    x = np.ascontiguousarray(np.asarray(x, dtype=np.float32))
    bias = np.asarray(bias, dtype=np.float32)

    wf = _gen_flipped_kernel(
        np.asarray(w1), np.asarray(b1), np.asarray(w2), np.asarray(b2),
        np.asarray(w3), np.asarray(b3),
    )  # [COUT, CIN, L]
    wds = _shuffle_weights(wf)

    if _NC is None:
        _NC = _build_nc()

    # host-side zero padding (4 + PAD cols) so the device needs no memset;
    # h=1 cores additionally see x shifted right by 16 (their taps are 16
    # later)
    xp0 = np.zeros((B, CIN, 4 + PAD + L), dtype=np.float32)
    xp0[:, :, 4 + PAD :] = x
    xp1 = np.zeros((B, CIN, 4 + PAD + L), dtype=np.float32)
    xp1[:, :, 4 + PAD + 16 :] = x[:, :, :-16]

    bias0 = np.ascontiguousarray(bias.reshape(COUT, 1))
    bias1 = np.zeros((COUT, 1), dtype=np.float32)

    in_maps = []
    for k in range(NCORES):
        b, h = k // 2, k % 2
        in_maps.append(
            {
                "xin": xp0[b] if h == 0 else xp1[b],
                "w": wds[h],
                "bias": bias0 if h == 0 else bias1,
            }
        )

    res = run_bass_kernel_spmd(_NC, in_maps, core_ids=list(range(NCORES)), trace=TRACE)
    LAST_RESULTS = res
    LAST_EXEC_NS = res.exec_time_ns

    out = np.empty((B, COUT, L), dtype=np.float32)
    for b in range(B):
        out[b] = res.results[2 * b]["out"] + res.results[2 * b + 1]["out"]
    return out


# revision 18
# speedup vs baseline: 1.1587x; 1.1587x over previous
"""CKConv (SIREN-generated causal conv1d) Trainium2 kernel.

Problem: x[B=4, Cin=32, L=2048]; a tiny SIREN MLP generates a conv kernel
[Cout=32, Cin=32, L]; output = causal conv + bias -> [4, 32, 2048].

Strategy:
  - Host: run the (negligible, O(H*L)) SIREN generator in numpy with
    REVERSED positions, producing the flipped kernel Wf[o,i,j'] directly
    (out[b,o,t] = sum_{i,j'<=t} Wf[o,i,j'] * x[b,i,t-j']), and pre-shuffle
    it into matmul tile layout.
  - Device (8 NeuronCores, SPMD): core k handles batch b=k//2 and the
    tap-parity half h=k%2 (alternating 16-tap blocks); the two cores of a
    pair produce partial sums the host adds (2M flops).  The causal conv
    is dense 128x128xN TensorE matmuls:
      K = 128 = (dj in 4) x (i in 32)   -- im2col: 4 time-shifted x copies
      M = 128 = (g in 4) x (o in 32)    -- 4 tap-groups per call
      N <= 512                          -- one PSUM bank per output tile
    Local call l covers taps j' = 32l + 16h + 4g + dj.  For output tile T
    (512 cols), calls l=0..16(T+1)-1 accumulate in PSUM; calls past the
    causal boundary are column-trimmed (their leading columns only touch
    zero padding).  Tap-group g lands shifted by 4g columns; VectorE folds
    the four 32-partition-aligned blocks into a [32, 2048] accumulator,
    ScalarE adds bias (h=0 core only), DMA out.
  - Weights stream just-in-time in 8 chunks; warmup matmuls during the
    initial DMA keep the PE HAM clock at 2.4 GHz.
"""

import numpy as np

import concourse.mybir as mybir
import concourse.tile as tile
from concourse import bacc
from concourse.bass_utils import run_bass_kernel_spmd

B, CIN, COUT, L, HID = 4, 32, 32, 2048, 32
OMEGA = 30.0
NCORES = 8
PAD = 512            # xim left zero padding (covers max lookback 16*31+496)
XIMW = PAD + L       # 2560
NT = 4               # output tiles of 512
NCH = 64             # weight tiles (local calls) per core; 16 taps each
NWCH = 8             # weight DMA chunks (8 calls each)
NWARM = 4            # PE warmup matmuls (f32, ~1-1.7us each)

KDTYPE = "fp16"      # "fp16" | "f32r"

TRACE = False
LAST_EXEC_NS = None
LAST_RESULTS = None

_NC = {}


def _build_nc(kdtype):
    nc = bacc.Bacc(None, target_bir_lowering=False)
    f32 = mybir.dt.float32
    dt = mybir.dt.float16 if kdtype == "fp16" else mybir.dt.float32r
    # x is host-padded: [4+PAD zero cols | x], so no on-device memset is
    # needed; the dj-th im2col block reads xin[:, 4-dj : 4-dj+XIMW].
    xin = nc.dram_tensor("xin", [CIN, 4 + XIMW], dt, kind="ExternalInput")
    wd = nc.dram_tensor("w", [128, NCH, 128], dt, kind="ExternalInput")
    bd = nc.dram_tensor("bias", [COUT, 1], f32, kind="ExternalInput")
    od = nc.dram_tensor("out", [COUT, L], f32, kind="ExternalOutput")

    with tile.TileContext(nc) as tc:
        with (
            tc.tile_pool(name="const", bufs=1) as cpool,
            tc.tile_pool(name="ps", bufs=2, space="PSUM") as pspool,
            tc.tile_pool(name="pswarm", bufs=1, space="PSUM") as pswarm,
            tc.tile_pool(name="stage", bufs=2) as spool,
        ):
            # PE warmup: f32 matmuls (4 cyc/row -> long) on a zeroed dummy
            # tile into a scratch PSUM bank that is never read.  No input
            # deps, so they run while the DMAs stream, lifting the HAM
            # clock gate to 2.4 GHz before the real matmuls start.
            dummy = cpool.tile([128, 512], f32)
            nc.vector.memset(dummy[:], 0.0)
            wps = pswarm.tile([128, 512], f32)
            for _ in range(NWARM):
                nc.tensor.matmul(
                    wps[:], dummy[:, 0:128], dummy[:], start=True, stop=True
                )

            # im2col input: xim[dj*32 + i, PAD + t + dj] = x[i, t]
            xim = cpool.tile([128, XIMW], dt)
            for dj in range(4):
                nc.sync.dma_start(
                    out=xim[dj * 32 : (dj + 1) * 32, :],
                    in_=xin[:, 4 - dj : 4 - dj + XIMW],
                )

            # weight tiles, NWCH chunks streamed just-in-time
            cs = NCH // NWCH
            wch = []
            for t in range(NWCH):
                wt = cpool.tile([128, cs, 128], dt, tag=f"w{t}")
                nc.sync.dma_start(out=wt[:], in_=wd[:, cs * t : cs * (t + 1), :])
                wch.append(wt)

            bias_sb = cpool.tile([COUT, 1], f32)
            nc.sync.dma_start(out=bias_sb[:], in_=bd[:])

            acc = cpool.tile([COUT, L + 64], f32)
            nc.vector.memset(acc[:], 0.0)

            for T in range(NT):
                ps = pspool.tile([128, 512], f32)
                ncalls = 16 * (T + 1)
                for l in range(ncalls):
                    s = PAD + 512 * T - 32 * l
                    # columns below n0 only touch the zero padding -> trim
                    n0 = max(0, 32 * (l - 16 * T))
                    nc.tensor.matmul(
                        ps[:, n0:512],
                        wch[l // cs][:, l % cs, :],
                        xim[:, s + n0 : s + 512],
                        start=(l == 0),
                        stop=(l == ncalls - 1),
                    )
                # fold tap-groups: psum[g*32+o, n] -> out[o, 512T + n + 4g]
                # (T=3's spill adds land in acc's pad columns, never read)
                for g in range(4):
                    nc.vector.tensor_add(
                        out=acc[:, 512 * T + 4 * g : 512 * T + 512],
                        in0=acc[:, 512 * T + 4 * g : 512 * T + 512],
                        in1=ps[32 * g : 32 * g + 32, 0 : 512 - 4 * g],
                    )
                for g in range(1, 4):
                    nc.vector.tensor_add(
                        out=acc[:, 512 * (T + 1) : 512 * (T + 1) + 4 * g],
                        in0=acc[:, 512 * (T + 1) : 512 * (T + 1) + 4 * g],
                        in1=ps[32 * g : 32 * g + 32, 512 - 4 * g : 512],
                    )
                ft = spool.tile([COUT, 512], f32, tag="ft")
                nc.scalar.activation(
                    ft[:],
                    acc[:, 512 * T : 512 * T + 512],
                    mybir.ActivationFunctionType.Identity,
                    bias=bias_sb[:],
                )
                nc.sync.dma_start(out=od[:, 512 * T : 512 * T + 512], in_=ft[:])

    nc.compile()
    return nc


def _gen_flipped_kernel(w1, b1, w2, b2, w3, b3):
    """SIREN generator with reversed positions -> Wf[o, i, j'] = k[o, i, L-1-j']."""
    pos = np.linspace(-1.0, 1.0, L, dtype=np.float32)[::-1].astype(np.float64)
    w1 = w1.astype(np.float64)
    w2 = w2.astype(np.float64)
    w3 = w3.astype(np.float64)
    h = np.sin(OMEGA * (w1[:, 0][:, None] * pos[None, :] + b1.astype(np.float64)[:, None]))
    h = np.sin(OMEGA * (w2 @ h + b2.astype(np.float64)[:, None]))
    k = w3 @ h + b3.astype(np.float64)[:, None]
    return k.reshape(COUT, CIN, L).astype(np.float32)


def _shuffle_weights(wf, npdt):
    """wf[o,i,j'] -> per tap-parity-half h: wt[p=dj*32+i, l, m=g*32+o]
    = wf[o, i, 32l + 16h + 4g + dj].

    The device pairs call l's weights with x-window column
    512T + n - 32l - dj and the unpack maps psum col n to
    out t = 512T + n + 4g.  With tap J = 32l + 16h + 4g + dj the correct
    x index is t - J = 512T + n - 32l - dj - 16h: the h=1 core therefore
    receives its input shifted right by 16 columns (see kernel()), which
    makes the device program identical on all cores.
    """
    outs = []
    for h in range(2):
        v = wf.reshape(COUT, CIN, NCH, 2, 4, 4)[:, :, :, h]   # [o,i,l,g,dj]
        v = v.transpose(4, 1, 2, 3, 0)                        # [dj,i,l,g,o]
        outs.append(np.ascontiguousarray(v.reshape(128, NCH, 128).astype(npdt)))
    return outs


def kernel(x, w1, b1, w2, b2, w3, b3, bias):
    global LAST_EXEC_NS, LAST_RESULTS
    x = np.ascontiguousarray(np.asarray(x, dtype=np.float32))
    bias = np.asarray(bias, dtype=np.float32)
    npdt = np.float16 if KDTYPE == "fp16" else np.float32

    wf = _gen_flipped_kernel(
        np.asarray(w1), np.asarray(b1), np.asarray(w2), np.asarray(b2),
        np.asarray(w3), np.asarray(b3),
    )  # [COUT, CIN, L]
    wds = _shuffle_weights(wf, npdt)

    if KDTYPE not in _NC:
        _NC[KDTYPE] = _build_nc(KDTYPE)

    # host-side zero padding (4 + PAD cols); h=1 cores see x shifted right
    # by 16 (their taps are 16 later)
    xp0 = np.zeros((B, CIN, 4 + XIMW), dtype=npdt)
    xp0[:, :, 4 + PAD :] = x.astype(npdt)
    xp1 = np.zeros((B, CIN, 4 + XIMW), dtype=npdt)
    xp1[:, :, 4 + PAD + 16 :] = x[:, :, :-16].astype(npdt)

    bias0 = np.ascontiguousarray(bias.reshape(COUT, 1))
    bias1 = np.zeros((COUT, 1), dtype=np.float32)

    in_maps = []
    for k in range(NCORES):
        b, h = k // 2, k % 2
        in_maps.append(
            {
                "xin": xp0[b] if h == 0 else xp1[b],
                "w": wds[h],
                "bias": bias0 if h == 0 else bias1,
            }
        )

    res = run_bass_kernel_spmd(
        _NC[KDTYPE], in_maps, core_ids=list(range(NCORES)), trace=TRACE
    )
    LAST_RESULTS = res
    LAST_EXEC_NS = res.exec_time_ns

    out = np.empty((B, COUT, L), dtype=np.float32)
    for b in range(B):
        out[b] = res.results[2 * b]["out"] + res.results[2 * b + 1]["out"]
    return out
